# revision 5
# baseline (speedup 1.0000x reference)
"""Child-Sum Tree-LSTM over a complete 4-ary tree on 8 TRN2 NeuronCores.

Tree: 21845 nodes, depth 7, branching 4. Leaves (level 7) keep h = c = 0, so
only the 5461 internal nodes produce output; rows 5461+ of h/c are zero.
Children of node j are 4j+1..4j+4 (contiguous), so with aligned block-sharding
per level each core's children lie in its own shard — at EVERY level down to
level 2 (core k owns l2 nodes {2k, 2k+1}, whose children are exactly its own
l3 slice). Levels 6..2 therefore run fully core-local with no communication;
only levels 1+0 (5 of 5461 nodes) cross shards, and those are finished on the
host in fp32 from the returned level-2/3 states. No collective in the NEFF.

On the axon/PJRT path, chained-execution latency is dominated by per-exec
host/transport overheads, not device time (~62 us/core by TimelineSim):
(a) every client-held ExternalInput operand is re-shipped each exec once
the chain feeds an output back as an input, (b) output-backing zero
operands cost the same as real inputs, and (c) the default bass_effect
token dispatch adds ~1 ms/exec of per-call sync. This kernel therefore:
  - bakes all weights into the NEFF as Const tensors (`inline_tensor`):
    DMA'd to HBM once at model load, zero per-exec wire cost (the baseline
    re-shipped 33.6 MB of replicated weights per exec);
  - keeps xT ([512, 682] fp16 per core) as the only runtime input and
    binds bass_exec WITHOUT output-backing zero operands (the kernel
    writes every output element, so PJRT-allocated result buffers are
    safe);
  - compiles the runner with fast_dispatch_compile (C++ fast-path
    dispatch, no effects token);
  - emits h/c into single contiguous SBUF regions, written out with one
    DMA per tensor ([512, 682] fp16 each).
Measured per-exec marginal (chained, 8 cores): ~0.23 ms vs ~12 ms for the
previous version.

On-device layout is transposed ([features, nodes]) so no transposes are
needed anywhere. Matmul operands (x, W, U, h) and intermediates are fp16
(PSUM accumulation fp32); per gate and feature chunk one PSUM group fuses
W@x (stride-0 broadcast replicates the parent's x across its 4 children for
the forget gate) with U@h_sum, and a single ScalarE activation applies
bias + nonlinearity straight out of PSUM. Child sums are DVE tensor_reduce
over a [*, m, 4] view.
"""

import numpy as np

_B = 4
_H = 512
_NN = 21845
_NI = 5461
_NC = 8
_OFFS = [0, 1, 5, 21, 85, 341, 1365, 5461, 21845]

_L2, _L3, _L4, _L5, _L6 = 2, 8, 32, 128, 512   # per-core slice sizes
_X2, _X3, _X4, _X5, _X6 = 0, 2, 10, 42, 170    # column offsets in xT
_NCOLS = _L2 + _L3 + _L4 + _L5 + _L6           # 682

_cache = {}


def _build_nc(wiouT, wfT, uiouT, ufT, biou_p, bf_p):
    """Build + compile the per-core NEFF with the weights baked in.

    wiouT/wfT/uiouT/ufT: transposed weights, fp16, shapes [512, 3H]/[512, H]/
    [512, 3H]/[512, H]. biou_p/bf_p: biases packed [128, 12]/[128, 4] fp32.
    """
    import concourse.bacc as bacc
    import concourse.tile as tile
    import concourse.mybir as mybir

    F32 = mybir.dt.float32
    F16 = mybir.dt.float16
    AF = mybir.ActivationFunctionType
    AXX = mybir.AxisListType.X

    nc = bacc.Bacc("TRN2", target_bir_lowering=False)

    xT = nc.declare_dram_parameter("xT", [_H, _NCOLS], F16, isOutput=False)
    oh = nc.declare_dram_parameter("oh", [_H, _NCOLS], F16, isOutput=True)
    oc = nc.declare_dram_parameter("oc", [_H, _NCOLS], F16, isOutput=True)

    c_wiou = nc.inline_tensor(np.asarray(wiouT, np.float16), name="c_wiou")
    c_wf = nc.inline_tensor(np.asarray(wfT, np.float16), name="c_wf")
    c_uiou = nc.inline_tensor(np.asarray(uiouT, np.float16), name="c_uiou")
    c_uf = nc.inline_tensor(np.asarray(ufT, np.float16), name="c_uf")
    c_biou = nc.inline_tensor(np.asarray(biou_p, np.float32), name="c_biou")
    c_bf = nc.inline_tensor(np.asarray(bf_p, np.float32), name="c_bf")

    def dview(p):
        # [F*128, n] DRAM -> [128, F, n] view (feature-chunk-major rows)
        return p.ap().rearrange("(f p) n -> p f n", p=128)

    with tile.TileContext(nc) as tc:
        with (
            tc.tile_pool(name="w", bufs=1) as wpool,
            tc.tile_pool(name="st", bufs=1) as spool,
            tc.tile_pool(name="tmp", bufs=2) as tpool,
            tc.tile_pool(name="psg", bufs=3, space="PSUM") as psg,
            tc.tile_pool(name="psu", bufs=2, space="PSUM") as psu,
        ):
            # --- weights / inputs to SBUF (k-chunk-major: [:, ck, :]) ---
            b_iou = wpool.tile([128, 12], F32)
            nc.sync.dma_start(out=b_iou[:], in_=c_biou.ap())
            b_f = wpool.tile([128, 4], F32)
            nc.sync.dma_start(out=b_f[:], in_=c_bf.ap())
            w_iou = wpool.tile([128, 4, 3 * _H], F16)
            xt = wpool.tile([128, 4, _NCOLS], F16)
            xt_v = dview(xT)
            # interleave per-chunk weight/x loads so the first l6 matmul's
            # operands (w_iou ck0 + xt ck0 l6-cols) arrive first
            for ck in range(4):
                nc.sync.dma_start(out=w_iou[:, ck, :],
                                  in_=dview(c_wiou)[:, ck, :])
                nc.sync.dma_start(out=xt[:, ck, _X6:], in_=xt_v[:, ck, _X6:])
            nc.sync.dma_start(out=xt[:, :, 0:_X6], in_=xt_v[:, :, 0:_X6])
            w_f = wpool.tile([128, 4, _H], F16)
            nc.sync.dma_start(out=w_f[:], in_=dview(c_wf))
            u_f = wpool.tile([128, 4, _H], F16)
            nc.sync.dma_start(out=u_f[:], in_=dview(c_uf))
            u_iou = wpool.tile([128, 4, 3 * _H], F16)
            nc.sync.dma_start(out=u_iou[:], in_=dview(c_uiou))

            # dummy sigmoid so the ACT function-table load (~1.3us) happens
            # during the DMA phase instead of stalling the first l6 gate
            warm = tpool.tile([128, 1], F16, tag="warm", name="warm", bufs=1)
            nc.vector.memset(warm[:], 0.0)
            nc.scalar.activation(warm[:], warm[:], AF.Sigmoid)

            # --- contiguous h/c stores covering all 682 per-core nodes ---
            h_all = spool.tile([128, 4, _NCOLS], F16, tag="h_all",
                               name="h_all")
            c_all = spool.tile([128, 4, _NCOLS], F16, tag="c_all",
                               name="c_all")

            # ---- level 6 (children are leaves: iou = wx only) ----
            # all 12 gate ACTs first so PSUM banks drain back-to-back; the
            # c/tanh/h tail then overlaps with l5's W-side matmuls
            g6 = {}
            for f in range(4):
                for g in range(3):  # 0=i 1=o 2=u
                    mt = 4 * g + f
                    ps = psg.tile([128, _L6], F32, tag="ps_g", name="ps")
                    for ck in range(4):
                        nc.tensor.matmul(ps[:],
                                         w_iou[:, ck, 128 * mt:128 * (mt + 1)],
                                         xt[:, ck, _X6:_X6 + _L6],
                                         start=(ck == 0), stop=(ck == 3))
                    gt = tpool.tile([128, _L6], F16, tag=f"g6_{g}",
                                    name=f"g6_{g}", bufs=4)
                    nc.scalar.activation(gt[:], ps[:],
                                         AF.Tanh if g == 2 else AF.Sigmoid,
                                         bias=b_iou[:, mt:mt + 1])
                    g6[(f, g)] = gt
            for f in range(4):
                cf = c_all[:, f, _X6:_X6 + _L6]
                nc.vector.tensor_mul(cf, g6[(f, 0)][:], g6[(f, 2)][:])
                tc6 = tpool.tile([128, _L6], F16, tag="tc6", name="tc6")
                nc.scalar.activation(tc6[:], cf, AF.Tanh)
                nc.vector.tensor_mul(h_all[:, f, _X6:_X6 + _L6],
                                     g6[(f, 1)][:], tc6[:])

            def level(m, xcol, ccol):
                """One internal level of m nodes at xT column xcol; its 4m
                children sit at columns ccol..ccol+4m of h_all/c_all."""
                hv = h_all[:, :, ccol:ccol + 4 * m].rearrange(
                    "p f (m k) -> p f m k", k=4)

                # forget side: psum = U_f @ h_ch + W_f @ x_parent (bcast x4);
                # prod = (psum + b_f) * c_ch, fc = sum over the 4 children
                prod = tpool.tile([128, 4, 4 * m], F16, tag="prod",
                                  name="prod")
                xb = [xt[:, ck, xcol:xcol + m].broadcast_to([128, m, 4])
                      for ck in range(4)]
                for f in range(4):
                    ps_uf = psu.tile([128, 4 * m], F32, tag="ps_f",
                                     name="ps_uf")
                    # W_f@x first: no dependency on the child level, so PE
                    # can fill these during the previous level's tail
                    puv = ps_uf[:].rearrange("p (m k) -> p m k", k=4)
                    for ck in range(4):
                        nc.tensor.matmul(puv,
                                         w_f[:, ck, 128 * f:128 * (f + 1)],
                                         xb[ck], start=(ck == 0), stop=False)
                    for ck in range(4):
                        nc.tensor.matmul(ps_uf[:],
                                         u_f[:, ck, 128 * f:128 * (f + 1)],
                                         h_all[:, ck, ccol:ccol + 4 * m],
                                         start=False, stop=(ck == 3))
                    nc.vector.scalar_tensor_tensor(
                        prod[:, f, :], ps_uf[:], b_f[:, f:f + 1],
                        c_all[:, f, ccol:ccol + 4 * m],
                        op0=mybir.AluOpType.add, op1=mybir.AluOpType.mult)
                pv = prod[:].rearrange("p f (m k) -> p f m k", k=4)
                fc = tpool.tile([128, 4, m], F16, tag="fc", name="fc")
                hsum = tpool.tile([128, 4, m], F16, tag="hsum", name="hsum")
                with nc.allow_low_precision("4-elt child sums"):
                    # per-chunk so each chunk's U matmuls start immediately
                    for ck in range(4):
                        nc.vector.tensor_reduce(hsum[:, ck, :], hv[:, ck],
                                                AXX, mybir.AluOpType.add)
                    for f in range(4):
                        nc.vector.tensor_reduce(fc[:, f, :], pv[:, f],
                                                AXX, mybir.AluOpType.add)

                # iou gates: psum = W @ x + U @ hsum, ACT+bias from PSUM;
                # pad so the i and o halves land in different PSUM banks
                ps_io = psg.tile([128, 2, 4, m], F32, tag="ps_io",
                                 name="ps_io", bufs=1,
                                 padded_shape=[128, 2, 4, 128])
                g_io = tpool.tile([128, 2, 4, m], F16, tag="g_io",
                                  name="g_io")
                ps_u = psg.tile([128, 4, m], F32, tag="ps_u", name="ps_u",
                                bufs=1)
                g_u = tpool.tile([128, 4, m], F16, tag="g_u", name="g_u")
                for g in range(3):
                    for f in range(4):
                        mt = 4 * g + f
                        sl = ps_u[:, f, :] if g == 2 else ps_io[:, g, f, :]
                        for ck in range(4):
                            nc.tensor.matmul(
                                sl, w_iou[:, ck, 128 * mt:128 * (mt + 1)],
                                xt[:, ck, xcol:xcol + m],
                                start=(ck == 0), stop=False)
                        for ck in range(4):
                            nc.tensor.matmul(
                                sl, u_iou[:, ck, 128 * mt:128 * (mt + 1)],
                                hsum[:, ck, :], start=False, stop=(ck == 3))
                        gt = g_u[:, f, :] if g == 2 else g_io[:, g, f, :]
                        nc.scalar.activation(gt, sl,
                                             AF.Tanh if g == 2 else AF.Sigmoid,
                                             bias=b_iou[:, mt:mt + 1])

                c_out = c_all[:, :, xcol:xcol + m]
                h_out = h_all[:, :, xcol:xcol + m]
                nc.vector.tensor_mul(c_out, g_io[:, 0], g_u[:])
                nc.vector.tensor_add(c_out, c_out, fc[:])
                tct = tpool.tile([128, 4, m], F16, tag="tct", name="tct")
                nc.scalar.activation(tct[:], c_out, AF.Tanh)
                nc.vector.tensor_mul(h_out, g_io[:, 1], tct[:])

            # ---- levels 5..2 (all core-local: children of a core's slice
            # at level l are exactly its slice at level l+1) ----
            level(_L5, _X5, _X6)
            level(_L4, _X4, _X5)
            level(_L3, _X3, _X4)
            level(_L2, _X2, _X3)

            # ---- outputs: one contiguous DMA per tensor ----
            nc.sync.dma_start(out=dview(oh), in_=h_all[:])
            nc.sync.dma_start(out=dview(oc), in_=c_all[:])

    nc.compile()
    return nc


def _core_rows(k):
    return np.concatenate([
        np.arange(_OFFS[2] + _L2 * k, _OFFS[2] + _L2 * (k + 1)),
        np.arange(_OFFS[3] + _L3 * k, _OFFS[3] + _L3 * (k + 1)),
        np.arange(_OFFS[4] + _L4 * k, _OFFS[4] + _L4 * (k + 1)),
        np.arange(_OFFS[5] + _L5 * k, _OFFS[5] + _L5 * (k + 1)),
        np.arange(_OFFS[6] + _L6 * k, _OFFS[6] + _L6 * (k + 1)),
    ])


def _pack_weights(W_iou, b_iou, W_f, b_f, U_iou, U_f):
    wiouT = np.ascontiguousarray(
        np.asarray(W_iou, np.float32).T).astype(np.float16)
    wfT = np.ascontiguousarray(
        np.asarray(W_f, np.float32).T).astype(np.float16)
    uiouT = np.ascontiguousarray(
        np.asarray(U_iou, np.float32).T).astype(np.float16)
    ufT = np.ascontiguousarray(
        np.asarray(U_f, np.float32).T).astype(np.float16)
    biou_p = np.ascontiguousarray(
        np.asarray(b_iou, np.float32).reshape(12, 128).T)
    bf_p = np.ascontiguousarray(
        np.asarray(b_f, np.float32).reshape(4, 128).T)
    return wiouT, wfT, uiouT, ufT, biou_p, bf_p


def _get_nc(*weights):
    key = tuple(np.asarray(w).tobytes() for w in weights)
    if _cache.get("wkey") != key:
        _cache.pop("runner", None)
        _cache["nc"] = _build_nc(*_pack_weights(*weights))
        _cache["wkey"] = key
    return _cache["nc"]


def _make_runner(nc):
    """Compile the SPMD module once and return a reusable callable."""
    import jax
    import numpy as _np
    from jax.experimental.shard_map import shard_map
    from jax.sharding import Mesh, PartitionSpec
    import concourse.mybir as mybir
    from concourse import bass2jax

    bass2jax.install_neuronx_cc_hook()

    partition_name = (nc.partition_id_tensor.name
                      if nc.partition_id_tensor else None)
    in_names, out_names, out_avals = [], [], []
    for alloc in nc.m.functions[0].allocations:
        if not isinstance(alloc, mybir.MemoryLocationSet):
            continue
        name = alloc.memorylocations[0].name
        if alloc.kind == "ExternalInput":
            if name != partition_name:
                in_names.append(name)
        elif alloc.kind == "ExternalOutput":
            shape = tuple(alloc.tensor_shape)
            dtype = mybir.dt.np(alloc.dtype)
            out_names.append(name)
            out_avals.append(jax.core.ShapedArray(shape, dtype))
    n_params = len(in_names)
    # outputs are NOT backed by zero operands: the kernel writes every
    # element of oh/oc, so PJRT-allocated (uninit) result buffers are fine
    # and we avoid shipping output-sized zeros per exec
    all_names = in_names
    if partition_name is not None:
        all_names = all_names + [partition_name]

    def _body(*args):
        operands = list(args)
        if partition_name is not None:
            operands.append(bass2jax.partition_id_tensor())
        outs = bass2jax._bass_exec_p.bind(
            *operands, out_avals=tuple(out_avals), in_names=tuple(all_names),
            out_names=tuple(out_names), lowering_input_output_aliases=(),
            sim_require_finite=True, sim_require_nnan=True, nc=nc)
        return tuple(outs)

    devices = jax.devices()[:_NC]
    mesh = Mesh(_np.asarray(devices), ("core",))
    smapped = shard_map(_body, mesh=mesh,
                        in_specs=(PartitionSpec("core"),) * n_params,
                        out_specs=(PartitionSpec("core"),) * len(out_names),
                        check_rep=False)
    compiled = {}

    def run(in_maps):
        concat_in = [_np.concatenate([m[k] for m in in_maps], axis=0)
                     for k in in_names]
        if "f" not in compiled:
            # C++ fast-path dispatch: suppresses the bass_effect token,
            # which otherwise adds ~1 ms/exec of per-call sync on the
            # axon path
            compiled["f"] = bass2jax.fast_dispatch_compile(
                lambda: jax.jit(smapped, keep_unused=True)
                .lower(*concat_in).compile())
        outs = compiled["f"](*concat_in)
        return [
            {name: _np.asarray(outs[i]).reshape(_NC, *out_avals[i].shape)[c]
             for i, name in enumerate(out_names)}
            for c in range(_NC)
        ]

    return run


def _get_runner():
    if "runner" not in _cache:
        _cache["runner"] = _make_runner(_cache["nc"])
    return _cache["runner"]


def _sigmoid(v):
    return 1.0 / (1.0 + np.exp(-v))


def kernel(x, children, W_iou, b_iou, W_f, b_f, U_iou, U_f):
    _get_nc(W_iou, b_iou, W_f, b_f, U_iou, U_f)
    run = _get_runner()

    x = np.asarray(x, dtype=np.float32)
    in_maps = []
    for k in range(_NC):
        xTk = np.ascontiguousarray(x[_core_rows(k)].T).astype(np.float16)
        in_maps.append({"xT": xTk})

    results = run(in_maps)

    h_full = np.zeros((_NN, _H), dtype=np.float32)
    c_full = np.zeros((_NN, _H), dtype=np.float32)
    for k in range(_NC):
        ohk = results[k]["oh"].astype(np.float32)
        ock = results[k]["oc"].astype(np.float32)
        for off, m, c0 in ((_OFFS[2], _L2, _X2), (_OFFS[3], _L3, _X3),
                           (_OFFS[4], _L4, _X4), (_OFFS[5], _L5, _X5),
                           (_OFFS[6], _L6, _X6)):
            h_full[off + m * k: off + m * (k + 1)] = ohk[:, c0:c0 + m].T
            c_full[off + m * k: off + m * (k + 1)] = ock[:, c0:c0 + m].T

    # ---- levels 1 and 0 (5 nodes) in fp32 on the host ----
    W_iou = np.asarray(W_iou, np.float32)
    b_iou = np.asarray(b_iou, np.float32)
    W_f = np.asarray(W_f, np.float32)
    b_f = np.asarray(b_f, np.float32)
    U_iou = np.asarray(U_iou, np.float32)
    U_f = np.asarray(U_f, np.float32)
    for s, e in ((1, 5), (0, 1)):
        ch = (4 * np.arange(s, e)[:, None]
              + np.arange(1, 5)[None, :])          # children indices [M, 4]
        h_ch = h_full[ch]                           # [M, 4, H]
        c_ch = c_full[ch]
        wx_iou = x[s:e] @ W_iou.T + b_iou
        wx_f = x[s:e] @ W_f.T + b_f
        iou = wx_iou + h_ch.sum(axis=1) @ U_iou.T
        i, o, u = np.split(iou, 3, axis=-1)
        i = _sigmoid(i)
        o = _sigmoid(o)
        u = np.tanh(u)
        f = wx_f[:, None, :] + h_ch @ U_f.T
        c_new = i * u + (f * c_ch).sum(axis=1)
        h_full[s:e] = o * np.tanh(c_new)
        c_full[s:e] = c_new
    return h_full, c_full


# revision 9
# speedup vs baseline: 1.0146x; 1.0146x over previous
"""Child-Sum Tree-LSTM over a complete 4-ary tree on 8 TRN2 NeuronCores.

Tree: 21845 nodes, depth 7, branching 4. Leaves (level 7) keep h = c = 0, so
only the 5461 internal nodes produce output; rows 5461+ of h/c are zero.
Children of node j are 4j+1..4j+4 (contiguous), so with aligned block-sharding
per level each core's children lie in its own shard — at EVERY level down to
level 2 (core k owns l2 nodes {2k, 2k+1}, whose children are exactly its own
l3 slice). Levels 6..2 therefore run fully core-local with no communication;
only levels 1+0 (5 of 5461 nodes) cross shards, and those are finished on the
host in fp32 from the returned level-2/3 states. No collective in the NEFF.

On the axon/PJRT path, chained-execution latency is dominated by per-exec
host/transport overheads, not device time (~62 us/core by TimelineSim):
(a) every client-held ExternalInput operand is re-shipped each exec once
the chain feeds an output back as an input, (b) output-backing zero
operands cost the same as real inputs, and (c) the default bass_effect
token dispatch adds ~1 ms/exec of per-call sync. This kernel therefore:
  - bakes all weights into the NEFF as Const tensors (`inline_tensor`):
    DMA'd to HBM once at model load, zero per-exec wire cost (the baseline
    re-shipped 33.6 MB of replicated weights per exec);
  - keeps xT ([512, 682] fp16 per core) as the only runtime input and
    binds bass_exec WITHOUT output-backing zero operands (the kernel
    writes every output element, so PJRT-allocated result buffers are
    safe);
  - compiles the runner with fast_dispatch_compile (C++ fast-path
    dispatch, no effects token);
  - emits h/c into single contiguous SBUF regions and streams each
    level's output slice as soon as it is final (h on the SP DMA ring,
    c on the ACT ring), overlapping the stores with upper-level compute.
Measured per-exec marginal (chained, 8 cores): ~0.23 ms vs ~12 ms for the
previous version.

On-device layout is transposed ([features, nodes]) so no transposes are
needed anywhere. Matmul operands (x, W, U, h) and intermediates are fp16
(PSUM accumulation fp32); per gate and feature chunk one PSUM group fuses
W@x (stride-0 broadcast replicates the parent's x across its 4 children for
the forget gate) with U@h_sum, and a single ScalarE activation applies
bias + nonlinearity straight out of PSUM. Child sums are DVE tensor_reduce
over a [*, m, 4] view.
"""

import numpy as np

_B = 4
_H = 512
_NN = 21845
_NI = 5461
_NC = 8
_OFFS = [0, 1, 5, 21, 85, 341, 1365, 5461, 21845]

_L2, _L3, _L4, _L5, _L6 = 2, 8, 32, 128, 512   # per-core slice sizes
_X2, _X3, _X4, _X5, _X6 = 0, 2, 10, 42, 170    # column offsets in xT
_NCOLS = _L2 + _L3 + _L4 + _L5 + _L6           # 682

_cache = {}


def _build_nc(wiouT, wfT, uiouT, ufT, biou_p, bf_p):
    """Build + compile the per-core NEFF with the weights baked in.

    wiouT/wfT/uiouT/ufT: transposed weights, fp16, shapes [512, 3H]/[512, H]/
    [512, 3H]/[512, H]. biou_p/bf_p: biases packed [128, 12]/[128, 4] fp32.
    """
    import concourse.bacc as bacc
    import concourse.tile as tile
    import concourse.mybir as mybir

    F32 = mybir.dt.float32
    F16 = mybir.dt.float16
    AF = mybir.ActivationFunctionType
    AXX = mybir.AxisListType.X

    nc = bacc.Bacc("TRN2", target_bir_lowering=False)

    xT = nc.declare_dram_parameter("xT", [_H, _NCOLS], F16, isOutput=False)
    oh = nc.declare_dram_parameter("oh", [_H, _NCOLS], F16, isOutput=True)
    oc = nc.declare_dram_parameter("oc", [_H, _NCOLS], F16, isOutput=True)

    c_wiou = nc.inline_tensor(np.asarray(wiouT, np.float16), name="c_wiou")
    c_wf = nc.inline_tensor(np.asarray(wfT, np.float16), name="c_wf")
    c_uiou = nc.inline_tensor(np.asarray(uiouT, np.float16), name="c_uiou")
    c_uf = nc.inline_tensor(np.asarray(ufT, np.float16), name="c_uf")
    c_biou = nc.inline_tensor(np.asarray(biou_p, np.float32), name="c_biou")
    c_bf = nc.inline_tensor(np.asarray(bf_p, np.float32), name="c_bf")

    def dview(p):
        # [F*128, n] DRAM -> [128, F, n] view (feature-chunk-major rows)
        return p.ap().rearrange("(f p) n -> p f n", p=128)

    with tile.TileContext(nc) as tc:
        with (
            tc.tile_pool(name="w", bufs=1) as wpool,
            tc.tile_pool(name="st", bufs=1) as spool,
            tc.tile_pool(name="tmp", bufs=2) as tpool,
            tc.tile_pool(name="psg", bufs=3, space="PSUM") as psg,
            tc.tile_pool(name="psu", bufs=2, space="PSUM") as psu,
        ):
            # --- weights / inputs to SBUF (k-chunk-major: [:, ck, :]) ---
            b_iou = wpool.tile([128, 12], F32)
            nc.sync.dma_start(out=b_iou[:], in_=c_biou.ap())
            b_f = wpool.tile([128, 4], F32)
            nc.sync.dma_start(out=b_f[:], in_=c_bf.ap())
            w_iou = wpool.tile([128, 4, 3 * _H], F16)
            xt = wpool.tile([128, 4, _NCOLS], F16)
            xt_v = dview(xT)
            # interleave per-chunk weight/x loads so the first l6 matmul's
            # operands (w_iou ck0 + xt ck0 l6-cols) arrive first
            for ck in range(4):
                nc.sync.dma_start(out=w_iou[:, ck, :],
                                  in_=dview(c_wiou)[:, ck, :])
                nc.sync.dma_start(out=xt[:, ck, _X6:], in_=xt_v[:, ck, _X6:])
            nc.sync.dma_start(out=xt[:, :, 0:_X6], in_=xt_v[:, :, 0:_X6])
            w_f = wpool.tile([128, 4, _H], F16)
            nc.sync.dma_start(out=w_f[:], in_=dview(c_wf))
            u_f = wpool.tile([128, 4, _H], F16)
            nc.sync.dma_start(out=u_f[:], in_=dview(c_uf))
            u_iou = wpool.tile([128, 4, 3 * _H], F16)
            nc.sync.dma_start(out=u_iou[:], in_=dview(c_uiou))

            # dummy sigmoid so the ACT function-table load (~1.3us) happens
            # during the DMA phase instead of stalling the first l6 gate
            warm = tpool.tile([128, 1], F16, tag="warm", name="warm", bufs=1)
            nc.vector.memset(warm[:], 0.0)
            nc.scalar.activation(warm[:], warm[:], AF.Sigmoid)

            # --- contiguous h/c stores covering all 682 per-core nodes ---
            h_all = spool.tile([128, 4, _NCOLS], F16, tag="h_all",
                               name="h_all")
            c_all = spool.tile([128, 4, _NCOLS], F16, tag="c_all",
                               name="c_all")

            # ---- level 6 (children are leaves: iou = wx only) ----
            # all 12 gate ACTs first so PSUM banks drain back-to-back; the
            # c/tanh/h tail then overlaps with l5's W-side matmuls
            g6 = {}
            for f in range(4):
                for g in range(3):  # 0=i 1=o 2=u
                    mt = 4 * g + f
                    ps = psg.tile([128, _L6], F32, tag="ps_g", name="ps")
                    for ck in range(4):
                        nc.tensor.matmul(ps[:],
                                         w_iou[:, ck, 128 * mt:128 * (mt + 1)],
                                         xt[:, ck, _X6:_X6 + _L6],
                                         start=(ck == 0), stop=(ck == 3))
                    gt = tpool.tile([128, _L6], F16, tag=f"g6_{g}",
                                    name=f"g6_{g}", bufs=4)
                    nc.scalar.activation(gt[:], ps[:],
                                         AF.Tanh if g == 2 else AF.Sigmoid,
                                         bias=b_iou[:, mt:mt + 1])
                    g6[(f, g)] = gt
            for f in range(4):
                cf = c_all[:, f, _X6:_X6 + _L6]
                nc.vector.tensor_mul(cf, g6[(f, 0)][:], g6[(f, 2)][:])
                tc6 = tpool.tile([128, _L6], F16, tag="tc6", name="tc6")
                nc.scalar.activation(tc6[:], cf, AF.Tanh)
                nc.vector.tensor_mul(h_all[:, f, _X6:_X6 + _L6],
                                     g6[(f, 1)][:], tc6[:])

            # stream each level's output as soon as it is final, h on the
            # SP ring and c on the ACT ring, so the stores (75% of output
            # bytes are l6's) overlap the upper levels' compute instead of
            # serializing after l2
            oh_v, oc_v = dview(oh), dview(oc)
            nc.sync.dma_start(out=oh_v[:, :, _X6:], in_=h_all[:, :, _X6:])
            nc.scalar.dma_start(out=oc_v[:, :, _X6:], in_=c_all[:, :, _X6:])

            def level(m, xcol, ccol):
                """One internal level of m nodes at xT column xcol; its 4m
                children sit at columns ccol..ccol+4m of h_all/c_all."""
                hv = h_all[:, :, ccol:ccol + 4 * m].rearrange(
                    "p f (m k) -> p f m k", k=4)

                # forget side: psum = U_f @ h_ch + W_f @ x_parent (bcast x4);
                # prod = (psum + b_f) * c_ch, fc = sum over the 4 children
                prod = tpool.tile([128, 4, 4 * m], F16, tag="prod",
                                  name="prod")
                xb = [xt[:, ck, xcol:xcol + m].broadcast_to([128, m, 4])
                      for ck in range(4)]
                for f in range(4):
                    ps_uf = psu.tile([128, 4 * m], F32, tag="ps_f",
                                     name="ps_uf")
                    # W_f@x first: no dependency on the child level, so PE
                    # can fill these during the previous level's tail
                    puv = ps_uf[:].rearrange("p (m k) -> p m k", k=4)
                    for ck in range(4):
                        nc.tensor.matmul(puv,
                                         w_f[:, ck, 128 * f:128 * (f + 1)],
                                         xb[ck], start=(ck == 0), stop=False)
                    for ck in range(4):
                        nc.tensor.matmul(ps_uf[:],
                                         u_f[:, ck, 128 * f:128 * (f + 1)],
                                         h_all[:, ck, ccol:ccol + 4 * m],
                                         start=False, stop=(ck == 3))
                    nc.vector.scalar_tensor_tensor(
                        prod[:, f, :], ps_uf[:], b_f[:, f:f + 1],
                        c_all[:, f, ccol:ccol + 4 * m],
                        op0=mybir.AluOpType.add, op1=mybir.AluOpType.mult)
                pv = prod[:].rearrange("p f (m k) -> p f m k", k=4)
                fc = tpool.tile([128, 4, m], F16, tag="fc", name="fc")
                hsum = tpool.tile([128, 4, m], F16, tag="hsum", name="hsum")
                with nc.allow_low_precision("4-elt child sums"):
                    # per-chunk so each chunk's U matmuls start immediately
                    for ck in range(4):
                        nc.vector.tensor_reduce(hsum[:, ck, :], hv[:, ck],
                                                AXX, mybir.AluOpType.add)
                    for f in range(4):
                        nc.vector.tensor_reduce(fc[:, f, :], pv[:, f],
                                                AXX, mybir.AluOpType.add)

                # iou gates: psum = W @ x + U @ hsum, ACT+bias from PSUM;
                # pad so the i and o halves land in different PSUM banks
                ps_io = psg.tile([128, 2, 4, m], F32, tag="ps_io",
                                 name="ps_io", bufs=1,
                                 padded_shape=[128, 2, 4, 128])
                g_io = tpool.tile([128, 2, 4, m], F16, tag="g_io",
                                  name="g_io")
                ps_u = psg.tile([128, 4, m], F32, tag="ps_u", name="ps_u",
                                bufs=1)
                g_u = tpool.tile([128, 4, m], F16, tag="g_u", name="g_u")
                for g in range(3):
                    for f in range(4):
                        mt = 4 * g + f
                        sl = ps_u[:, f, :] if g == 2 else ps_io[:, g, f, :]
                        for ck in range(4):
                            nc.tensor.matmul(
                                sl, w_iou[:, ck, 128 * mt:128 * (mt + 1)],
                                xt[:, ck, xcol:xcol + m],
                                start=(ck == 0), stop=False)
                        for ck in range(4):
                            nc.tensor.matmul(
                                sl, u_iou[:, ck, 128 * mt:128 * (mt + 1)],
                                hsum[:, ck, :], start=False, stop=(ck == 3))
                        gt = g_u[:, f, :] if g == 2 else g_io[:, g, f, :]
                        nc.scalar.activation(gt, sl,
                                             AF.Tanh if g == 2 else AF.Sigmoid,
                                             bias=b_iou[:, mt:mt + 1])

                c_out = c_all[:, :, xcol:xcol + m]
                h_out = h_all[:, :, xcol:xcol + m]
                nc.vector.tensor_mul(c_out, g_io[:, 0], g_u[:])
                nc.vector.tensor_add(c_out, c_out, fc[:])
                tct = tpool.tile([128, 4, m], F16, tag="tct", name="tct")
                nc.scalar.activation(tct[:], c_out, AF.Tanh)
                nc.vector.tensor_mul(h_out, g_io[:, 1], tct[:])
                nc.sync.dma_start(out=oh_v[:, :, xcol:xcol + m], in_=h_out)
                nc.scalar.dma_start(out=oc_v[:, :, xcol:xcol + m], in_=c_out)

            # ---- levels 5..2 (all core-local: children of a core's slice
            # at level l are exactly its slice at level l+1) ----
            level(_L5, _X5, _X6)
            level(_L4, _X4, _X5)
            level(_L3, _X3, _X4)
            level(_L2, _X2, _X3)

    nc.compile()
    return nc


def _core_rows(k):
    return np.concatenate([
        np.arange(_OFFS[2] + _L2 * k, _OFFS[2] + _L2 * (k + 1)),
        np.arange(_OFFS[3] + _L3 * k, _OFFS[3] + _L3 * (k + 1)),
        np.arange(_OFFS[4] + _L4 * k, _OFFS[4] + _L4 * (k + 1)),
        np.arange(_OFFS[5] + _L5 * k, _OFFS[5] + _L5 * (k + 1)),
        np.arange(_OFFS[6] + _L6 * k, _OFFS[6] + _L6 * (k + 1)),
    ])


def _pack_weights(W_iou, b_iou, W_f, b_f, U_iou, U_f):
    wiouT = np.ascontiguousarray(
        np.asarray(W_iou, np.float32).T).astype(np.float16)
    wfT = np.ascontiguousarray(
        np.asarray(W_f, np.float32).T).astype(np.float16)
    uiouT = np.ascontiguousarray(
        np.asarray(U_iou, np.float32).T).astype(np.float16)
    ufT = np.ascontiguousarray(
        np.asarray(U_f, np.float32).T).astype(np.float16)
    biou_p = np.ascontiguousarray(
        np.asarray(b_iou, np.float32).reshape(12, 128).T)
    bf_p = np.ascontiguousarray(
        np.asarray(b_f, np.float32).reshape(4, 128).T)
    return wiouT, wfT, uiouT, ufT, biou_p, bf_p


def _get_nc(*weights):
    key = tuple(np.asarray(w).tobytes() for w in weights)
    if _cache.get("wkey") != key:
        _cache.pop("runner", None)
        _cache["nc"] = _build_nc(*_pack_weights(*weights))
        _cache["wkey"] = key
    return _cache["nc"]


def _make_runner(nc):
    """Compile the SPMD module once and return a reusable callable."""
    import jax
    import numpy as _np
    from jax.experimental.shard_map import shard_map
    from jax.sharding import Mesh, PartitionSpec
    import concourse.mybir as mybir
    from concourse import bass2jax

    bass2jax.install_neuronx_cc_hook()

    partition_name = (nc.partition_id_tensor.name
                      if nc.partition_id_tensor else None)
    in_names, out_names, out_avals = [], [], []
    for alloc in nc.m.functions[0].allocations:
        if not isinstance(alloc, mybir.MemoryLocationSet):
            continue
        name = alloc.memorylocations[0].name
        if alloc.kind == "ExternalInput":
            if name != partition_name:
                in_names.append(name)
        elif alloc.kind == "ExternalOutput":
            shape = tuple(alloc.tensor_shape)
            dtype = mybir.dt.np(alloc.dtype)
            out_names.append(name)
            out_avals.append(jax.core.ShapedArray(shape, dtype))
    n_params = len(in_names)
    # outputs are NOT backed by zero operands: the kernel writes every
    # element of oh/oc, so PJRT-allocated (uninit) result buffers are fine
    # and we avoid shipping output-sized zeros per exec
    all_names = in_names
    if partition_name is not None:
        all_names = all_names + [partition_name]

    def _body(*args):
        operands = list(args)
        if partition_name is not None:
            operands.append(bass2jax.partition_id_tensor())
        outs = bass2jax._bass_exec_p.bind(
            *operands, out_avals=tuple(out_avals), in_names=tuple(all_names),
            out_names=tuple(out_names), lowering_input_output_aliases=(),
            sim_require_finite=True, sim_require_nnan=True, nc=nc)
        return tuple(outs)

    devices = jax.devices()[:_NC]
    mesh = Mesh(_np.asarray(devices), ("core",))
    smapped = shard_map(_body, mesh=mesh,
                        in_specs=(PartitionSpec("core"),) * n_params,
                        out_specs=(PartitionSpec("core"),) * len(out_names),
                        check_rep=False)
    compiled = {}

    def run(in_maps):
        concat_in = [_np.concatenate([m[k] for m in in_maps], axis=0)
                     for k in in_names]
        if "f" not in compiled:
            # C++ fast-path dispatch: suppresses the bass_effect token,
            # which otherwise adds ~1 ms/exec of per-call sync on the
            # axon path
            compiled["f"] = bass2jax.fast_dispatch_compile(
                lambda: jax.jit(smapped, keep_unused=True)
                .lower(*concat_in).compile())
        outs = compiled["f"](*concat_in)
        return [
            {name: _np.asarray(outs[i]).reshape(_NC, *out_avals[i].shape)[c]
             for i, name in enumerate(out_names)}
            for c in range(_NC)
        ]

    return run


def _get_runner():
    if "runner" not in _cache:
        _cache["runner"] = _make_runner(_cache["nc"])
    return _cache["runner"]


def _sigmoid(v):
    return 1.0 / (1.0 + np.exp(-v))


def kernel(x, children, W_iou, b_iou, W_f, b_f, U_iou, U_f):
    _get_nc(W_iou, b_iou, W_f, b_f, U_iou, U_f)
    run = _get_runner()

    x = np.asarray(x, dtype=np.float32)
    in_maps = []
    for k in range(_NC):
        xTk = np.ascontiguousarray(x[_core_rows(k)].T).astype(np.float16)
        in_maps.append({"xT": xTk})

    results = run(in_maps)

    h_full = np.zeros((_NN, _H), dtype=np.float32)
    c_full = np.zeros((_NN, _H), dtype=np.float32)
    for k in range(_NC):
        ohk = results[k]["oh"].astype(np.float32)
        ock = results[k]["oc"].astype(np.float32)
        for off, m, c0 in ((_OFFS[2], _L2, _X2), (_OFFS[3], _L3, _X3),
                           (_OFFS[4], _L4, _X4), (_OFFS[5], _L5, _X5),
                           (_OFFS[6], _L6, _X6)):
            h_full[off + m * k: off + m * (k + 1)] = ohk[:, c0:c0 + m].T
            c_full[off + m * k: off + m * (k + 1)] = ock[:, c0:c0 + m].T

    # ---- levels 1 and 0 (5 nodes) in fp32 on the host ----
    W_iou = np.asarray(W_iou, np.float32)
    b_iou = np.asarray(b_iou, np.float32)
    W_f = np.asarray(W_f, np.float32)
    b_f = np.asarray(b_f, np.float32)
    U_iou = np.asarray(U_iou, np.float32)
    U_f = np.asarray(U_f, np.float32)
    for s, e in ((1, 5), (0, 1)):
        ch = (4 * np.arange(s, e)[:, None]
              + np.arange(1, 5)[None, :])          # children indices [M, 4]
        h_ch = h_full[ch]                           # [M, 4, H]
        c_ch = c_full[ch]
        wx_iou = x[s:e] @ W_iou.T + b_iou
        wx_f = x[s:e] @ W_f.T + b_f
        iou = wx_iou + h_ch.sum(axis=1) @ U_iou.T
        i, o, u = np.split(iou, 3, axis=-1)
        i = _sigmoid(i)
        o = _sigmoid(o)
        u = np.tanh(u)
        f = wx_f[:, None, :] + h_ch @ U_f.T
        c_new = i * u + (f * c_ch).sum(axis=1)
        h_full[s:e] = o * np.tanh(c_new)
        c_full[s:e] = c_new
    return h_full, c_full
